# revision 1
# baseline (speedup 1.0000x reference)
"""Trainium2 Bass kernel v6: v5 + wave-split normalization.

Same GEMV/softmax design as v5 (see kernel.py docstring), plus: slots
whose columns complete early (wave A, all but the last ~24 columns) get
their entire normalization chain -- cross-partition sum, reciprocal,
broadcast, selection-matrix expand, multiply, and output DMA -- emitted
mid-stream where it hides under the remaining input DMA.  Only the few
last slots (wave B) run their (latency-bound) chain after the final
input group, so the exposed tail shrinks from ~6 us to ~3 us.
"""

import time

import numpy as np

EMBED = 512
LMAX = 2048
NCORES = 8
B2 = 256
SEQS = B2 // NCORES        # 32 sequences per core, one per column-slot
TILE = 128                 # tokens per PSUM column (= stationary cols)
GCOLS = 16                 # columns per DMA group / PSUM bank (2 MB fp16)
QBUFS = 4                  # input tile buffering depth
PSBUFS = 4                 # rotating PSUM banks for the GEMV
QDT = "f16"                # wire dtype: "f16" or "f8" (e4m3)
WAVE_TAIL = 12             # target wave-B width in columns
EMIT_SLACK = 2             # groups between wave-A data ready and chain emit

_nc_cache = {}


def _schedule(lens):
    """Sort rows by tile count, deal round-robin to cores; every core's
    slot j holds K[j] = max tile count of the 8 rows in that slot."""
    k = (lens + TILE - 1) // TILE            # [256] tiles per row (>=1)
    order = np.argsort(-k, kind="stable")
    K = np.empty(SEQS, np.int64)
    for j in range(SEQS):
        K[j] = k[order[j * NCORES:(j + 1) * NCORES]].max()
    B = np.zeros(SEQS, np.int64)
    B[1:] = np.cumsum(K)[:-1]
    T = int(B[-1] + K[-1])
    Tcols = ((T + 3) // 4) * 4
    assert Tcols <= 512, "one PSUM bank holds <= 512 fp32 columns"
    return order, K, B, Tcols


def _groups(Tcols):
    """Group sizes: GCOLS-wide, tapering to 4 at the end so the PE work
    exposed after the last DMA is small."""
    sizes = []
    rest = Tcols
    taper = [8, 4]
    while rest > sum(taper) + GCOLS - 1:
        sizes.append(GCOLS)
        rest -= GCOLS
    while rest >= 4:
        for t in taper:
            if rest >= t + (4 if t > 4 else 0) or rest == t:
                sizes.append(t)
                rest -= t
                break
        else:
            sizes.append(4)
            rest -= 4
    assert rest == 0 and sum(sizes) == Tcols, (sizes, Tcols)
    return sizes


def _split(K, B, Tcols):
    """Wave split: jA = count of slots ending at or before Tcols-WAVE_TAIL;
    CA = first wave-B column.  Guarantees 1 <= jA < SEQS."""
    ends = B + K
    jA = int(np.searchsorted(ends, Tcols - WAVE_TAIL, side="right"))
    jA = max(1, min(SEQS - 1, jA))
    CA = int(B[jA])
    return jA, CA


def _build_nc(Tcols, K, B):
    from contextlib import ExitStack

    import concourse.bass as bass
    import concourse.tile as tile
    from concourse import bacc, mybir

    fq = mybir.dt.float8e4 if QDT == "f8" else mybir.dt.float16
    f32 = mybir.dt.float32
    Ttok = Tcols * TILE
    sizes = _groups(Tcols)
    starts = np.zeros(len(sizes), np.int64)
    starts[1:] = np.cumsum(sizes)[:-1]
    ngrp = len(sizes)
    jA, CA = _split(K, B, Tcols)
    nB = SEQS - jA

    # slot j's columns are complete after the group containing B[j]+K[j]-1
    reduces_after = {g: [] for g in range(ngrp)}
    for j in range(SEQS):
        last = int(B[j]) + int(K[j]) - 1
        g = int(np.searchsorted(starts, last, side="right")) - 1
        reduces_after[g].append(j)
    # emit wave-A chain two groups after the one exp'ing column CA-1
    gA = int(np.searchsorted(starts, CA - 1, side="right")) - 1
    gA_emit = min(gA + EMIT_SLACK, ngrp - 1)

    nc = bacc.Bacc("TRN2", target_bir_lowering=False, debug=False,
                   num_devices=NCORES)
    q_h = nc.dram_tensor("qpt", [4, 128, Ttok], fq, kind="ExternalInput")
    w_h = nc.dram_tensor("w", [128, 4], fq, kind="ExternalInput")
    e_h = nc.dram_tensor("esel", [SEQS, Tcols], f32, kind="ExternalInput")
    eb_h = nc.dram_tensor("eselb", [SEQS, Tcols], f32, kind="ExternalInput")
    out_h = nc.dram_tensor("out", [128, Tcols], f32, kind="ExternalOutput")

    with tile.TileContext(nc) as tc, ExitStack() as ctx:
        singles = ctx.enter_context(tc.tile_pool(name="singles", bufs=1))
        qpool = ctx.enter_context(tc.tile_pool(name="qpool", bufs=QBUFS))
        psum = ctx.enter_context(tc.tile_pool(name="psum", bufs=PSBUFS,
                                              space="PSUM"))
        psum1 = ctx.enter_context(tc.tile_pool(name="psum1", bufs=1,
                                               space="PSUM"))

        w_sb = singles.tile([128, 4], fq)
        nc.sync.dma_start(out=w_sb, in_=w_h.ap())
        e_sb = singles.tile([SEQS, Tcols], f32)
        nc.sync.dma_start(out=e_sb, in_=e_h.ap())
        eb_sb = singles.tile([SEQS, Tcols], f32)
        nc.sync.dma_start(out=eb_sb, in_=eb_h.ap())
        ones_k = singles.tile([128, 1], f32)
        nc.vector.memset(ones_k, 1.0)
        ones_m = singles.tile([1, 128], f32)
        nc.vector.memset(ones_m, 1.0)

        expm = singles.tile([128, Tcols], f32)
        sums = singles.tile([128, SEQS], f32)
        rec = singles.tile([1, SEQS], f32)
        recbT = singles.tile([SEQS, 128], f32)
        recbTB = singles.tile([SEQS, 128], f32)
        outt = singles.tile([128, Tcols], f32)
        s_ps = psum1.tile([1, SEQS], f32, tag="s_ps")
        r2_ps = psum1.tile([SEQS, 128], f32, tag="r2_ps")
        r2b_ps = psum1.tile([SEQS, 128], f32, tag="r2b_ps")
        sc_ps = psum1.tile([128, Tcols], f32, tag="sc_ps")

        def chain_a():
            nc.tensor.matmul(s_ps[:, :jA], ones_k, sums[:, :jA],
                             start=True, stop=True)
            nc.vector.reciprocal(rec[:, :jA], s_ps[:, :jA])
            nc.tensor.matmul(r2_ps[:jA, :], rec[:, :jA], ones_m,
                             start=True, stop=True)
            nc.vector.tensor_copy(recbT[:jA, :], r2_ps[:jA, :])
            nc.tensor.matmul(sc_ps[:, :CA], recbT[:jA, :], e_sb[:jA, :CA],
                             start=True, stop=True)
            nc.vector.tensor_mul(outt[:, :CA], expm[:, :CA], sc_ps[:, :CA])
            nc.sync.dma_start(out=out_h.ap()[:, :CA], in_=outt[:, :CA])

        # ---- GEMV + pipelined exp/rowsum, wave-A chain mid-stream
        for g in range(ngrp):
            g0, gn = int(starts[g]), int(sizes[g])
            qt = qpool.tile([128, 4, gn * TILE], fq, tag=f"qt{gn}")
            nc.sync.dma_start(
                out=qt,
                in_=bass.AP(tensor=q_h, offset=g0 * TILE,
                            ap=[[Ttok, 128], [128 * Ttok, 4],
                                [1, gn * TILE]]))
            e_ps = psum.tile([128, 512], f32, tag="eps")
            for tt in range(gn):
                for c in range(4):
                    nc.tensor.matmul(e_ps[:, tt:tt + 1],
                                     qt[:, c, tt * TILE:(tt + 1) * TILE],
                                     w_sb[:, c:c + 1],
                                     start=(c == 0), stop=(c == 3))
            nc.scalar.activation(out=expm[:, g0:g0 + gn],
                                 in_=e_ps[:, :gn],
                                 func=mybir.ActivationFunctionType.Exp)
            for j in reduces_after[g]:
                nc.vector.tensor_reduce(out=sums[:, j:j + 1],
                                        in_=expm[:, B[j]:B[j] + K[j]],
                                        axis=mybir.AxisListType.X,
                                        op=mybir.AluOpType.add)
            if g == gA_emit:
                chain_a()
        if gA_emit >= ngrp:  # pragma: no cover (gA_emit clamped above)
            chain_a()

        # ---- wave B: remaining slots, columns [CA, Tcols)
        nc.tensor.matmul(s_ps[:, jA:], ones_k, sums[:, jA:],
                         start=True, stop=True)
        nc.vector.reciprocal(rec[:, jA:], s_ps[:, jA:])
        nc.tensor.matmul(r2b_ps[:nB, :], rec[:, jA:], ones_m,
                         start=True, stop=True)
        nc.vector.tensor_copy(recbTB[:nB, :], r2b_ps[:nB, :])
        nc.tensor.matmul(sc_ps[:, CA:], recbTB[:nB, :], eb_sb[:nB, CA:],
                         start=True, stop=True)
        nc.vector.tensor_mul(outt[:, CA:], expm[:, CA:], sc_ps[:, CA:])
        nc.sync.dma_start(out=out_h.ap()[:, CA:], in_=outt[:, CA:])

    nc.compile()
    return nc


def _get_nc(Tcols, K, B):
    key = (Tcols, tuple(int(x) for x in K))
    if key not in _nc_cache:
        _nc_cache[key] = _build_nc(Tcols, K, B)
    return _nc_cache[key]


def prepare(questions, questions_lens, lin_w, weight_vec):
    """Host-side sharding: schedule, fold W into u, pack/cast/transpose."""
    import ml_dtypes

    q = np.asarray(questions)
    lens = np.asarray(questions_lens).astype(np.int64).ravel()
    w = np.asarray(lin_w, dtype=np.float64)
    v = np.asarray(weight_vec, dtype=np.float64)
    u = (w.T @ v).astype(np.float32)

    order, K, B, Tcols = _schedule(lens)
    jA, CA = _split(K, B, Tcols)
    Ttok = Tcols * TILE
    npdt = ml_dtypes.float8_e4m3 if QDT == "f8" else np.float16
    unorm = float(u.astype(np.float64) @ u.astype(np.float64))
    pad_tok = ((-60.0 / unorm) * u).astype(npdt)   # energy ~ -60 -> exp ~ 0
    w_sb = np.ascontiguousarray(
        u.reshape(4, 128).T.astype(npdt))          # w_sb[p, c] = u[c*128+p]
    esel = np.zeros((SEQS, Tcols), np.float32)
    for j in range(SEQS):
        esel[j, B[j]:B[j] + K[j]] = 1.0
    eselb = np.zeros((SEQS, Tcols), np.float32)
    eselb[:SEQS - jA] = esel[jA:]                  # wave-B rows, shifted to 0

    in_maps = []
    for c in range(NCORES):
        buf = np.empty((Ttok, EMBED), npdt)
        buf[:] = pad_tok
        for j in range(SEQS):
            r = order[j * NCORES + c]
            n = int(lens[r])
            buf[B[j] * TILE:B[j] * TILE + n] = q[r, :n]
        qpt = np.ascontiguousarray(buf.T).reshape(4, 128, Ttok)
        in_maps.append({"qpt": qpt, "w": w_sb, "esel": esel, "eselb": eselb})
    return in_maps, (order, K, B, Tcols, lens)


def unpack(core_outs, meta):
    order, K, B, Tcols, lens = meta
    full = np.zeros((B2, LMAX), np.float32)
    for c in range(NCORES):
        o = np.asarray(core_outs[c])                 # [128, Tcols]
        for j in range(SEQS):
            r = order[j * NCORES + c]
            n = int(lens[r])
            blk = o[:, B[j]:B[j] + K[j]]             # [128 tok, K_j tiles]
            full[r, :n] = blk.T.reshape(-1)[:n]
    return full


def run_sharded(questions, questions_lens, lin_w, lin_b, weight_vec,
                trace=False):
    """Shard across the 8 cores, run, gather.  Returns (out, results)."""
    from concourse.bass_utils import run_bass_kernel_spmd

    in_maps, meta = prepare(questions, questions_lens, lin_w, weight_vec)
    nc = _get_nc(meta[3], meta[1], meta[2])

    res = None
    last_err = None
    for attempt in range(5):
        try:
            res = run_bass_kernel_spmd(nc, in_maps,
                                       core_ids=list(range(NCORES)),
                                       trace=trace)
            break
        except ModuleNotFoundError:
            trace = False
            continue
        except Exception as e:  # device left unrecoverable by a prior crash
            last_err = e
            if "UNAVAILABLE" in str(e) or "UNRECOVERABLE" in str(e):
                time.sleep(20 * (attempt + 1))
                continue
            raise
    if res is None:
        raise last_err
    out = unpack([r["out"] for r in res.results], meta)
    return out, res


def kernel(questions, questions_lens, lin_w, lin_b, weight_vec):
    out, _ = run_sharded(questions, questions_lens, lin_w, lin_b, weight_vec)
    return out



# revision 9
# speedup vs baseline: 1.8091x; 1.8091x over previous
"""Trainium2 Bass kernel v7: v6 + float8 e3m4 wire format.

Same GEMV/softmax design as v6 (wave-split normalization), but the
questions tensor travels to the device as float8_e3m4 (4 mantissa
bits), halving HBM traffic vs fp16 -- the kernel is DMA-bound, so this
roughly halves exec time.  Numerics: q is scaled by QS=2 and u by a
power of two US so both sit in e3m4's normal range [0.25, 15.5); the
folded weight vector is sent as an e3m4 (hi, lo) pair per embed chunk
(8 accumulating matmuls per 128-token tile instead of 4) so weight
quantization error is second-order; the Exp activation un-scales with
scale=1/(QS*US) (exact, power of two).
"""

import time

import numpy as np

EMBED = 512
LMAX = 2048
NCORES = 8
B2 = 256
SEQS = B2 // NCORES        # 32 sequences per core, one per column-slot
TILE = 128                 # tokens per PSUM column (= stationary cols)
GCOLS = 16                 # columns per DMA group / PSUM bank
QBUFS = 4                  # input tile buffering depth
PSBUFS = 4                 # rotating PSUM banks for the GEMV
QDT = "f8e3"               # wire dtype: "f16", "f8" (e4m3) or "f8e3" (e3m4)
QS = 2.0                   # host-side scale on q before e3m4 cast
F8MAX = 15.49              # e3m4 saturation guard (true max 15.5 -> inf)
WAVE_TAIL = 12             # target wave-B width in columns
EMIT_SLACK = 2             # groups between wave-A data ready and chain emit

_nc_cache = {}


def _schedule(lens):
    """Sort rows by tile count, deal round-robin to cores; every core's
    slot j holds K[j] = max tile count of the 8 rows in that slot."""
    k = (lens + TILE - 1) // TILE            # [256] tiles per row (>=1)
    order = np.argsort(-k, kind="stable")
    K = np.empty(SEQS, np.int64)
    for j in range(SEQS):
        K[j] = k[order[j * NCORES:(j + 1) * NCORES]].max()
    B = np.zeros(SEQS, np.int64)
    B[1:] = np.cumsum(K)[:-1]
    T = int(B[-1] + K[-1])
    Tcols = ((T + 3) // 4) * 4
    assert Tcols <= 512, "one PSUM bank holds <= 512 fp32 columns"
    return order, K, B, Tcols


def _groups(Tcols):
    """Group sizes: GCOLS-wide, tapering to 4 at the end so the PE work
    exposed after the last DMA is small."""
    sizes = []
    rest = Tcols
    taper = [8, 4]
    while rest > sum(taper) + GCOLS - 1:
        sizes.append(GCOLS)
        rest -= GCOLS
    while rest >= 4:
        for t in taper:
            if rest >= t + (4 if t > 4 else 0) or rest == t:
                sizes.append(t)
                rest -= t
                break
        else:
            sizes.append(4)
            rest -= 4
    assert rest == 0 and sum(sizes) == Tcols, (sizes, Tcols)
    return sizes


def _split(K, B, Tcols):
    """Wave split: jA = count of slots ending at or before Tcols-WAVE_TAIL;
    CA = first wave-B column.  Guarantees 1 <= jA < SEQS."""
    ends = B + K
    jA = int(np.searchsorted(ends, Tcols - WAVE_TAIL, side="right"))
    jA = max(1, min(SEQS - 1, jA))
    CA = int(B[jA])
    return jA, CA


def _build_nc(Tcols, K, B, scale=1.0):
    from contextlib import ExitStack

    import concourse.bass as bass
    import concourse.tile as tile
    from concourse import bacc, mybir

    fq = {"f8": mybir.dt.float8e4, "f8e3": mybir.dt.float8e3,
          "f16": mybir.dt.float16}[QDT]
    wcols = 8 if QDT == "f8e3" else 4    # e3m4 sends u as a (hi, lo) pair
    f32 = mybir.dt.float32
    Ttok = Tcols * TILE
    sizes = _groups(Tcols)
    starts = np.zeros(len(sizes), np.int64)
    starts[1:] = np.cumsum(sizes)[:-1]
    ngrp = len(sizes)
    jA, CA = _split(K, B, Tcols)
    nB = SEQS - jA

    # slot j's columns are complete after the group containing B[j]+K[j]-1
    reduces_after = {g: [] for g in range(ngrp)}
    for j in range(SEQS):
        last = int(B[j]) + int(K[j]) - 1
        g = int(np.searchsorted(starts, last, side="right")) - 1
        reduces_after[g].append(j)
    # emit wave-A chain two groups after the one exp'ing column CA-1
    gA = int(np.searchsorted(starts, CA - 1, side="right")) - 1
    gA_emit = min(gA + EMIT_SLACK, ngrp - 1)

    nc = bacc.Bacc("TRN2", target_bir_lowering=False, debug=False,
                   num_devices=NCORES)
    q_h = nc.dram_tensor("qpt", [4, 128, Ttok], fq, kind="ExternalInput")
    w_h = nc.dram_tensor("w", [128, wcols], fq, kind="ExternalInput")
    e_h = nc.dram_tensor("esel", [SEQS, Tcols], f32, kind="ExternalInput")
    eb_h = nc.dram_tensor("eselb", [SEQS, Tcols], f32, kind="ExternalInput")
    out_h = nc.dram_tensor("out", [128, Tcols], f32, kind="ExternalOutput")

    with tile.TileContext(nc) as tc, ExitStack() as ctx:
        singles = ctx.enter_context(tc.tile_pool(name="singles", bufs=1))
        qpool = ctx.enter_context(tc.tile_pool(name="qpool", bufs=QBUFS))
        psum = ctx.enter_context(tc.tile_pool(name="psum", bufs=PSBUFS,
                                              space="PSUM"))
        psum1 = ctx.enter_context(tc.tile_pool(name="psum1", bufs=1,
                                               space="PSUM"))

        w_sb = singles.tile([128, wcols], fq)
        nc.sync.dma_start(out=w_sb, in_=w_h.ap())
        e_sb = singles.tile([SEQS, Tcols], f32)
        nc.sync.dma_start(out=e_sb, in_=e_h.ap())
        eb_sb = singles.tile([SEQS, Tcols], f32)
        nc.sync.dma_start(out=eb_sb, in_=eb_h.ap())
        ones_k = singles.tile([128, 1], f32)
        nc.vector.memset(ones_k, 1.0)
        ones_m = singles.tile([1, 128], f32)
        nc.vector.memset(ones_m, 1.0)

        expm = singles.tile([128, Tcols], f32)
        sums = singles.tile([128, SEQS], f32)
        rec = singles.tile([1, SEQS], f32)
        recbT = singles.tile([SEQS, 128], f32)
        recbTB = singles.tile([SEQS, 128], f32)
        outt = singles.tile([128, Tcols], f32)
        s_ps = psum1.tile([1, SEQS], f32, tag="s_ps")
        r2_ps = psum1.tile([SEQS, 128], f32, tag="r2_ps")
        r2b_ps = psum1.tile([SEQS, 128], f32, tag="r2b_ps")
        sc_ps = psum1.tile([128, Tcols], f32, tag="sc_ps")

        def chain_a():
            nc.tensor.matmul(s_ps[:, :jA], ones_k, sums[:, :jA],
                             start=True, stop=True)
            nc.vector.reciprocal(rec[:, :jA], s_ps[:, :jA])
            nc.tensor.matmul(r2_ps[:jA, :], rec[:, :jA], ones_m,
                             start=True, stop=True)
            nc.vector.tensor_copy(recbT[:jA, :], r2_ps[:jA, :])
            nc.tensor.matmul(sc_ps[:, :CA], recbT[:jA, :], e_sb[:jA, :CA],
                             start=True, stop=True)
            nc.vector.tensor_mul(outt[:, :CA], expm[:, :CA], sc_ps[:, :CA])
            nc.sync.dma_start(out=out_h.ap()[:, :CA], in_=outt[:, :CA])

        # ---- GEMV + pipelined exp/rowsum, wave-A chain mid-stream
        for g in range(ngrp):
            g0, gn = int(starts[g]), int(sizes[g])
            qt = qpool.tile([128, 4, gn * TILE], fq, tag=f"qt{gn}")
            nc.sync.dma_start(
                out=qt,
                in_=bass.AP(tensor=q_h, offset=g0 * TILE,
                            ap=[[Ttok, 128], [128 * Ttok, 4],
                                [1, gn * TILE]]))
            e_ps = psum.tile([128, 512], f32, tag="eps")
            for tt in range(gn):
                nmm = 0
                for c in range(4):
                    for wc in range(c, wcols, 4):  # hi (and lo) of chunk c
                        nmm += 1
                        nc.tensor.matmul(e_ps[:, tt:tt + 1],
                                         qt[:, c, tt * TILE:(tt + 1) * TILE],
                                         w_sb[:, wc:wc + 1],
                                         start=(nmm == 1), stop=(nmm == wcols))
            nc.scalar.activation(out=expm[:, g0:g0 + gn],
                                 in_=e_ps[:, :gn],
                                 func=mybir.ActivationFunctionType.Exp,
                                 scale=float(scale))
            for j in reduces_after[g]:
                nc.vector.tensor_reduce(out=sums[:, j:j + 1],
                                        in_=expm[:, B[j]:B[j] + K[j]],
                                        axis=mybir.AxisListType.X,
                                        op=mybir.AluOpType.add)
            if g == gA_emit:
                chain_a()
        if gA_emit >= ngrp:  # pragma: no cover (gA_emit clamped above)
            chain_a()

        # ---- wave B: remaining slots, columns [CA, Tcols)
        nc.tensor.matmul(s_ps[:, jA:], ones_k, sums[:, jA:],
                         start=True, stop=True)
        nc.vector.reciprocal(rec[:, jA:], s_ps[:, jA:])
        nc.tensor.matmul(r2b_ps[:nB, :], rec[:, jA:], ones_m,
                         start=True, stop=True)
        nc.vector.tensor_copy(recbTB[:nB, :], r2b_ps[:nB, :])
        nc.tensor.matmul(sc_ps[:, CA:], recbTB[:nB, :], eb_sb[:nB, CA:],
                         start=True, stop=True)
        nc.vector.tensor_mul(outt[:, CA:], expm[:, CA:], sc_ps[:, CA:])
        nc.sync.dma_start(out=out_h.ap()[:, CA:], in_=outt[:, CA:])

    nc.compile()
    return nc


def _get_nc(Tcols, K, B, scale=1.0):
    key = (Tcols, tuple(int(x) for x in K), float(scale))
    if key not in _nc_cache:
        _nc_cache[key] = _build_nc(Tcols, K, B, scale)
    return _nc_cache[key]


def prepare(questions, questions_lens, lin_w, weight_vec):
    """Host-side sharding: schedule, fold W into u, pack/cast/transpose."""
    import ml_dtypes

    q = np.asarray(questions)
    lens = np.asarray(questions_lens).astype(np.int64).ravel()
    w = np.asarray(lin_w, dtype=np.float64)
    v = np.asarray(weight_vec, dtype=np.float64)
    u = (w.T @ v).astype(np.float32)

    order, K, B, Tcols = _schedule(lens)
    jA, CA = _split(K, B, Tcols)
    Ttok = Tcols * TILE
    unorm = float(u.astype(np.float64) @ u.astype(np.float64))
    if QDT == "f8e3":
        npdt = ml_dtypes.float8_e3m4
        # u scaled by a power of two into e3m4's normal range, sent as a
        # hi+lo pair (residual also e3m4, same scale) -> u error is ~2nd
        # order.  q scaled by QS=2; energies come out scaled by QS*US and
        # the Exp activation un-scales exactly.
        us = 2.0 ** int(np.floor(np.log2(F8MAX / np.abs(u).max())))
        u_s = (u * us).astype(np.float32)
        u_hi = u_s.astype(npdt)
        u_lo = np.clip(u_s - u_hi.astype(np.float32),
                       -F8MAX, F8MAX).astype(npdt)
        scale = 1.0 / (QS * us)
        # pad token: energy ~ -35 (exp ~ 6e-16), capped so QS*pad stays
        # inside e3m4 range
        alpha = min(35.0 / unorm, F8MAX / (QS * float(np.abs(u).max())))
        pad_tok = (-alpha * QS * u).astype(npdt)
        e_pad = float(pad_tok.astype(np.float64)
                      @ (u_hi.astype(np.float64) + u_lo.astype(np.float64))
                      ) * scale
        assert e_pad < -25.0, e_pad
        w_sb = np.ascontiguousarray(np.concatenate(
            [u_hi.reshape(4, 128).T, u_lo.reshape(4, 128).T], axis=1))
        qmax = float(np.abs(q).max())
        qclip = qmax * QS > F8MAX
    else:
        npdt = ml_dtypes.float8_e4m3 if QDT == "f8" else np.float16
        scale = 1.0
        pad_tok = ((-60.0 / unorm) * u).astype(npdt)
        w_sb = np.ascontiguousarray(
            u.reshape(4, 128).T.astype(npdt))      # w_sb[p, c] = u[c*128+p]
    esel = np.zeros((SEQS, Tcols), np.float32)
    for j in range(SEQS):
        esel[j, B[j]:B[j] + K[j]] = 1.0
    eselb = np.zeros((SEQS, Tcols), np.float32)
    eselb[:SEQS - jA] = esel[jA:]                  # wave-B rows, shifted to 0

    in_maps = []
    for c in range(NCORES):
        buf = np.empty((Ttok, EMBED), npdt)
        buf[:] = pad_tok
        for j in range(SEQS):
            r = order[j * NCORES + c]
            n = int(lens[r])
            if QDT == "f8e3":
                row = q[r, :n] * QS
                if qclip:
                    row = np.clip(row, -F8MAX, F8MAX)
                buf[B[j] * TILE:B[j] * TILE + n] = row
            else:
                buf[B[j] * TILE:B[j] * TILE + n] = q[r, :n]
        qpt = np.ascontiguousarray(buf.T).reshape(4, 128, Ttok)
        in_maps.append({"qpt": qpt, "w": w_sb, "esel": esel, "eselb": eselb})
    return in_maps, (order, K, B, Tcols, lens, scale)


def unpack(core_outs, meta):
    order, K, B, Tcols, lens, _scale = meta
    full = np.zeros((B2, LMAX), np.float32)
    for c in range(NCORES):
        o = np.asarray(core_outs[c])                 # [128, Tcols]
        for j in range(SEQS):
            r = order[j * NCORES + c]
            n = int(lens[r])
            blk = o[:, B[j]:B[j] + K[j]]             # [128 tok, K_j tiles]
            full[r, :n] = blk.T.reshape(-1)[:n]
    return full


def run_sharded(questions, questions_lens, lin_w, lin_b, weight_vec,
                trace=False):
    """Shard across the 8 cores, run, gather.  Returns (out, results)."""
    from concourse.bass_utils import run_bass_kernel_spmd

    in_maps, meta = prepare(questions, questions_lens, lin_w, weight_vec)
    nc = _get_nc(meta[3], meta[1], meta[2], meta[5])

    res = None
    last_err = None
    for attempt in range(5):
        try:
            res = run_bass_kernel_spmd(nc, in_maps,
                                       core_ids=list(range(NCORES)),
                                       trace=trace)
            break
        except ModuleNotFoundError:
            trace = False
            continue
        except Exception as e:  # device left unrecoverable by a prior crash
            last_err = e
            if "UNAVAILABLE" in str(e) or "UNRECOVERABLE" in str(e):
                time.sleep(20 * (attempt + 1))
                continue
            raise
    if res is None:
        raise last_err
    out = unpack([r["out"] for r in res.results], meta)
    return out, res


def kernel(questions, questions_lens, lin_w, lin_b, weight_vec):
    out, _ = run_sharded(questions, questions_lens, lin_w, lin_b, weight_vec)
    return out



# revision 17
# speedup vs baseline: 1.8165x; 1.0041x over previous
"""Trainium2 Bass kernel v7: v6 + float8 e3m4 wire format.

Same GEMV/softmax design as v6 (wave-split normalization), but the
questions tensor travels to the device as float8_e3m4 (4 mantissa
bits), halving HBM traffic vs fp16 -- the kernel is DMA-bound, so this
roughly halves exec time.  Numerics: q is scaled by QS=2 and u by a
power of two US so both sit in e3m4's normal range [0.25, 15.5); the
folded weight vector is sent as an e3m4 (hi, lo) pair per embed chunk
(8 accumulating matmuls per 128-token tile instead of 4) so weight
quantization error is second-order; the Exp activation un-scales with
scale=1/(QS*US) (exact, power of two).
"""

import time

import numpy as np

EMBED = 512
LMAX = 2048
NCORES = 8
B2 = 256
SEQS = B2 // NCORES        # 32 sequences per core, one per column-slot
TILE = 128                 # tokens per PSUM column (= stationary cols)
GCOLS = 16                 # columns per DMA group / PSUM bank
QBUFS = 4                  # input tile buffering depth
PSBUFS = 4                 # rotating PSUM banks for the GEMV
QDT = "f8e3"               # wire dtype: "f16", "f8" (e4m3) or "f8e3" (e3m4)
QS = 2.0                   # host-side scale on q before e3m4 cast
F8MAX = 15.49              # e3m4 saturation guard (true max 15.5 -> inf)
WAVE_TAIL = 12             # target wave-B width in columns
EMIT_SLACK = 2             # groups between wave-A data ready and chain emit

_nc_cache = {}


def _schedule(lens):
    """Sort rows by tile count, deal round-robin to cores; every core's
    slot j holds K[j] = max tile count of the 8 rows in that slot."""
    k = (lens + TILE - 1) // TILE            # [256] tiles per row (>=1)
    order = np.argsort(-k, kind="stable")
    K = np.empty(SEQS, np.int64)
    for j in range(SEQS):
        K[j] = k[order[j * NCORES:(j + 1) * NCORES]].max()
    B = np.zeros(SEQS, np.int64)
    B[1:] = np.cumsum(K)[:-1]
    T = int(B[-1] + K[-1])
    Tcols = ((T + 3) // 4) * 4
    assert Tcols <= 512, "one PSUM bank holds <= 512 fp32 columns"
    return order, K, B, Tcols


def _groups(Tcols):
    """Group sizes: GCOLS-wide, then a 17-31 column taper of 8s/4s at the
    end.  The wave-A chain is emitted at the first taper group and hides
    under the taper's DMA stream; smaller tail groups also shrink the
    instruction-queue drain after the last transfer.  Groups stay >= 4
    cols so the DMA's contiguous run is >= 512 B (no 2x cost penalty)."""
    sizes = []
    rest = Tcols
    while rest > 2 * GCOLS:
        sizes.append(GCOLS)
        rest -= GCOLS
    n16 = len(sizes)
    while rest > 8:
        sizes.append(8)
        rest -= 8
    if rest:
        sizes.append(rest)   # 4 or 8 (Tcols is a multiple of 4)
    assert sum(sizes) == Tcols and all(s >= 4 for s in sizes), (sizes, Tcols)
    return sizes, n16


def _split(K, B, boundary):
    """Wave split: jA = count of slots whose columns all land before
    `boundary` (the first taper-group column); CA = first wave-B column.
    Guarantees 1 <= jA < SEQS."""
    ends = B + K
    jA = int(np.searchsorted(ends, boundary, side="right"))
    jA = max(1, min(SEQS - 1, jA))
    CA = int(B[jA])
    return jA, CA


def _build_nc(Tcols, K, B, scale=1.0):
    from contextlib import ExitStack

    import concourse.bass as bass
    import concourse.tile as tile
    from concourse import bacc, mybir

    fq = {"f8": mybir.dt.float8e4, "f8e3": mybir.dt.float8e3,
          "f16": mybir.dt.float16}[QDT]
    wcols = 8 if QDT == "f8e3" else 4    # e3m4 sends u as a (hi, lo) pair
    f32 = mybir.dt.float32
    Ttok = Tcols * TILE
    sizes, n16 = _groups(Tcols)
    starts = np.zeros(len(sizes), np.int64)
    starts[1:] = np.cumsum(sizes)[:-1]
    ngrp = len(sizes)
    jA, CA = _split(K, B, int(starts[n16]) if 0 < n16 < ngrp else Tcols)
    nB = SEQS - jA

    # slot j's columns are complete after the group containing B[j]+K[j]-1
    reduces_after = {g: [] for g in range(ngrp)}
    for j in range(SEQS):
        last = int(B[j]) + int(K[j]) - 1
        g = int(np.searchsorted(starts, last, side="right")) - 1
        reduces_after[g].append(j)
    # emit the wave-A chain at the first taper group: its inputs (slots
    # ending before the taper) are ready, and it hides under the taper DMA
    gA_emit = min(n16, ngrp - 1)

    nc = bacc.Bacc("TRN2", target_bir_lowering=False, debug=False,
                   num_devices=NCORES)
    q_h = nc.dram_tensor("qpt", [4, 128, Ttok], fq, kind="ExternalInput")
    w_h = nc.dram_tensor("w", [128, wcols], fq, kind="ExternalInput")
    e_h = nc.dram_tensor("esel", [SEQS, Tcols], f32, kind="ExternalInput")
    eb_h = nc.dram_tensor("eselb", [SEQS, Tcols], f32, kind="ExternalInput")
    out_h = nc.dram_tensor("out", [128, Tcols], f32, kind="ExternalOutput")

    with tile.TileContext(nc) as tc, ExitStack() as ctx:
        singles = ctx.enter_context(tc.tile_pool(name="singles", bufs=1))
        qpool = ctx.enter_context(tc.tile_pool(name="qpool", bufs=QBUFS))
        psum = ctx.enter_context(tc.tile_pool(name="psum", bufs=PSBUFS,
                                              space="PSUM"))
        psum1 = ctx.enter_context(tc.tile_pool(name="psum1", bufs=1,
                                               space="PSUM"))

        # group 0's input DMA goes first so the big stream owns the DMA
        # engines from the start; the small singles hide under it
        qt_first = qpool.tile([128, 4, int(sizes[0]) * TILE], fq,
                              tag=f"qt{int(sizes[0])}")
        nc.sync.dma_start(
            out=qt_first,
            in_=bass.AP(tensor=q_h, offset=0,
                        ap=[[Ttok, 128], [128 * Ttok, 4],
                            [1, int(sizes[0]) * TILE]]))
        w_sb = singles.tile([128, wcols], fq)
        nc.sync.dma_start(out=w_sb, in_=w_h.ap())
        e_sb = singles.tile([SEQS, Tcols], f32)
        nc.sync.dma_start(out=e_sb, in_=e_h.ap())
        eb_sb = singles.tile([SEQS, Tcols], f32)
        nc.sync.dma_start(out=eb_sb, in_=eb_h.ap())
        ones_k = singles.tile([128, 1], f32)
        nc.vector.memset(ones_k, 1.0)
        ones_m = singles.tile([1, 128], f32)
        nc.vector.memset(ones_m, 1.0)

        expm = singles.tile([128, Tcols], f32)
        sums = singles.tile([128, SEQS], f32)
        rec = singles.tile([1, SEQS], f32)
        recbT = singles.tile([SEQS, 128], f32)
        recbTB = singles.tile([SEQS, 128], f32)
        outt = singles.tile([128, Tcols], f32)
        s_ps = psum1.tile([1, SEQS], f32, tag="s_ps")
        r2_ps = psum1.tile([SEQS, 128], f32, tag="r2_ps")
        r2b_ps = psum1.tile([SEQS, 128], f32, tag="r2b_ps")
        sc_ps = psum1.tile([128, Tcols], f32, tag="sc_ps")

        def chain_a():
            nc.tensor.matmul(s_ps[:, :jA], ones_k, sums[:, :jA],
                             start=True, stop=True)
            nc.vector.reciprocal(rec[:, :jA], s_ps[:, :jA])
            nc.tensor.matmul(r2_ps[:jA, :], rec[:, :jA], ones_m,
                             start=True, stop=True)
            nc.vector.tensor_copy(recbT[:jA, :], r2_ps[:jA, :])
            nc.tensor.matmul(sc_ps[:, :CA], recbT[:jA, :], e_sb[:jA, :CA],
                             start=True, stop=True)
            nc.vector.tensor_mul(outt[:, :CA], expm[:, :CA], sc_ps[:, :CA])
            nc.sync.dma_start(out=out_h.ap()[:, :CA], in_=outt[:, :CA])

        # ---- GEMV + pipelined exp/rowsum, wave-A chain mid-stream
        for g in range(ngrp):
            g0, gn = int(starts[g]), int(sizes[g])
            if g == 0:
                qt = qt_first
            else:
                qt = qpool.tile([128, 4, gn * TILE], fq, tag=f"qt{gn}")
                nc.sync.dma_start(
                    out=qt,
                    in_=bass.AP(tensor=q_h, offset=g0 * TILE,
                                ap=[[Ttok, 128], [128 * Ttok, 4],
                                    [1, gn * TILE]]))
            if g == gA_emit:
                chain_a()
            e_ps = psum.tile([128, 512], f32, tag="eps")
            for tt in range(gn):
                nmm = 0
                for c in range(4):
                    for wc in range(c, wcols, 4):  # hi (and lo) of chunk c
                        nmm += 1
                        nc.tensor.matmul(e_ps[:, tt:tt + 1],
                                         qt[:, c, tt * TILE:(tt + 1) * TILE],
                                         w_sb[:, wc:wc + 1],
                                         start=(nmm == 1), stop=(nmm == wcols))
            nc.scalar.activation(out=expm[:, g0:g0 + gn],
                                 in_=e_ps[:, :gn],
                                 func=mybir.ActivationFunctionType.Exp,
                                 scale=float(scale))
            for j in reduces_after[g]:
                nc.vector.tensor_reduce(out=sums[:, j:j + 1],
                                        in_=expm[:, B[j]:B[j] + K[j]],
                                        axis=mybir.AxisListType.X,
                                        op=mybir.AluOpType.add)

        # ---- wave B: remaining slots, columns [CA, Tcols)
        nc.tensor.matmul(s_ps[:, jA:], ones_k, sums[:, jA:],
                         start=True, stop=True)
        nc.vector.reciprocal(rec[:, jA:], s_ps[:, jA:])
        nc.tensor.matmul(r2b_ps[:nB, :], rec[:, jA:], ones_m,
                         start=True, stop=True)
        nc.vector.tensor_copy(recbTB[:nB, :], r2b_ps[:nB, :])
        nc.tensor.matmul(sc_ps[:, CA:], recbTB[:nB, :], eb_sb[:nB, CA:],
                         start=True, stop=True)
        nc.vector.tensor_mul(outt[:, CA:], expm[:, CA:], sc_ps[:, CA:])
        nc.sync.dma_start(out=out_h.ap()[:, CA:], in_=outt[:, CA:])

    nc.compile()
    return nc


def _get_nc(Tcols, K, B, scale=1.0):
    key = (Tcols, tuple(int(x) for x in K), float(scale))
    if key not in _nc_cache:
        _nc_cache[key] = _build_nc(Tcols, K, B, scale)
    return _nc_cache[key]


def prepare(questions, questions_lens, lin_w, weight_vec):
    """Host-side sharding: schedule, fold W into u, pack/cast/transpose."""
    import ml_dtypes

    q = np.asarray(questions)
    lens = np.asarray(questions_lens).astype(np.int64).ravel()
    w = np.asarray(lin_w, dtype=np.float64)
    v = np.asarray(weight_vec, dtype=np.float64)
    u = (w.T @ v).astype(np.float32)

    order, K, B, Tcols = _schedule(lens)
    sizes, n16 = _groups(Tcols)
    boundary = (int(np.cumsum(sizes)[n16 - 1])
                if 0 < n16 < len(sizes) else Tcols)
    jA, CA = _split(K, B, boundary)
    Ttok = Tcols * TILE
    unorm = float(u.astype(np.float64) @ u.astype(np.float64))
    if QDT == "f8e3":
        npdt = ml_dtypes.float8_e3m4
        # u scaled by a power of two into e3m4's normal range, sent as a
        # hi+lo pair (residual also e3m4, same scale) -> u error is ~2nd
        # order.  q scaled by QS=2; energies come out scaled by QS*US and
        # the Exp activation un-scales exactly.
        us = 2.0 ** int(np.floor(np.log2(F8MAX / np.abs(u).max())))
        u_s = (u * us).astype(np.float32)
        u_hi = u_s.astype(npdt)
        u_lo = np.clip(u_s - u_hi.astype(np.float32),
                       -F8MAX, F8MAX).astype(npdt)
        scale = 1.0 / (QS * us)
        # pad token: energy ~ -35 (exp ~ 6e-16), capped so QS*pad stays
        # inside e3m4 range
        alpha = min(35.0 / unorm, F8MAX / (QS * float(np.abs(u).max())))
        pad_tok = (-alpha * QS * u).astype(npdt)
        e_pad = float(pad_tok.astype(np.float64)
                      @ (u_hi.astype(np.float64) + u_lo.astype(np.float64))
                      ) * scale
        assert e_pad < -25.0, e_pad
        w_sb = np.ascontiguousarray(np.concatenate(
            [u_hi.reshape(4, 128).T, u_lo.reshape(4, 128).T], axis=1))
        qmax = float(np.abs(q).max())
        qclip = qmax * QS > F8MAX
    else:
        npdt = ml_dtypes.float8_e4m3 if QDT == "f8" else np.float16
        scale = 1.0
        pad_tok = ((-60.0 / unorm) * u).astype(npdt)
        w_sb = np.ascontiguousarray(
            u.reshape(4, 128).T.astype(npdt))      # w_sb[p, c] = u[c*128+p]
    esel = np.zeros((SEQS, Tcols), np.float32)
    for j in range(SEQS):
        esel[j, B[j]:B[j] + K[j]] = 1.0
    eselb = np.zeros((SEQS, Tcols), np.float32)
    eselb[:SEQS - jA] = esel[jA:]                  # wave-B rows, shifted to 0

    in_maps = []
    for c in range(NCORES):
        buf = np.empty((Ttok, EMBED), npdt)
        buf[:] = pad_tok
        for j in range(SEQS):
            r = order[j * NCORES + c]
            n = int(lens[r])
            if QDT == "f8e3":
                row = q[r, :n] * QS
                if qclip:
                    row = np.clip(row, -F8MAX, F8MAX)
                buf[B[j] * TILE:B[j] * TILE + n] = row
            else:
                buf[B[j] * TILE:B[j] * TILE + n] = q[r, :n]
        qpt = np.ascontiguousarray(buf.T).reshape(4, 128, Ttok)
        in_maps.append({"qpt": qpt, "w": w_sb, "esel": esel, "eselb": eselb})
    return in_maps, (order, K, B, Tcols, lens, scale)


def unpack(core_outs, meta):
    order, K, B, Tcols, lens, _scale = meta
    full = np.zeros((B2, LMAX), np.float32)
    for c in range(NCORES):
        o = np.asarray(core_outs[c])                 # [128, Tcols]
        for j in range(SEQS):
            r = order[j * NCORES + c]
            n = int(lens[r])
            blk = o[:, B[j]:B[j] + K[j]]             # [128 tok, K_j tiles]
            full[r, :n] = blk.T.reshape(-1)[:n]
    return full


def run_sharded(questions, questions_lens, lin_w, lin_b, weight_vec,
                trace=False):
    """Shard across the 8 cores, run, gather.  Returns (out, results)."""
    from concourse.bass_utils import run_bass_kernel_spmd

    in_maps, meta = prepare(questions, questions_lens, lin_w, weight_vec)
    nc = _get_nc(meta[3], meta[1], meta[2], meta[5])

    res = None
    last_err = None
    for attempt in range(5):
        try:
            res = run_bass_kernel_spmd(nc, in_maps,
                                       core_ids=list(range(NCORES)),
                                       trace=trace)
            break
        except ModuleNotFoundError:
            trace = False
            continue
        except Exception as e:  # device left unrecoverable by a prior crash
            last_err = e
            if "UNAVAILABLE" in str(e) or "UNRECOVERABLE" in str(e):
                time.sleep(20 * (attempt + 1))
                continue
            raise
    if res is None:
        raise last_err
    out = unpack([r["out"] for r in res.results], meta)
    return out, res


def kernel(questions, questions_lens, lin_w, lin_b, weight_vec):
    out, _ = run_sharded(questions, questions_lens, lin_w, lin_b, weight_vec)
    return out



# revision 20
# speedup vs baseline: 1.8775x; 1.0336x over previous
"""Trainium2 Bass kernel v7: v6 + float8 e3m4 wire format.

Same GEMV/softmax design as v6 (wave-split normalization), but the
questions tensor travels to the device as float8_e3m4 (4 mantissa
bits), halving HBM traffic vs fp16 -- the kernel is DMA-bound, so this
roughly halves exec time.  Numerics: q is scaled by QS=2 and u by a
power of two US so both sit in e3m4's normal range [0.25, 15.5); the
folded weight vector is sent as an e3m4 (hi, lo) pair per embed chunk
(8 accumulating matmuls per 128-token tile instead of 4) so weight
quantization error is second-order; the Exp activation un-scales with
scale=1/(QS*US) (exact, power of two).
"""

import time

import numpy as np

EMBED = 512
LMAX = 2048
NCORES = 8
B2 = 256
SEQS = B2 // NCORES        # 32 sequences per core, one per column-slot
TILE = 128                 # tokens per PSUM column (= stationary cols)
GCOLS = 16                 # columns per DMA group / PSUM bank
QBUFS = 4                  # input tile buffering depth
PSBUFS = 4                 # rotating PSUM banks for the GEMV
QDT = "f8e3"               # wire dtype: "f16", "f8" (e4m3) or "f8e3" (e3m4)
QS = 2.0                   # host-side scale on q before e3m4 cast
F8MAX = 15.49              # e3m4 saturation guard (true max 15.5 -> inf)
WAVE_TAIL = 12             # target wave-B width in columns
EMIT_SLACK = 2             # groups between wave-A data ready and chain emit

_nc_cache = {}


def _schedule(lens):
    """Sort rows by tile count, deal round-robin to cores; every core's
    slot j holds K[j] = max tile count of the 8 rows in that slot."""
    k = (lens + TILE - 1) // TILE            # [256] tiles per row (>=1)
    order = np.argsort(-k, kind="stable")
    K = np.empty(SEQS, np.int64)
    for j in range(SEQS):
        K[j] = k[order[j * NCORES:(j + 1) * NCORES]].max()
    B = np.zeros(SEQS, np.int64)
    B[1:] = np.cumsum(K)[:-1]
    T = int(B[-1] + K[-1])
    Tcols = ((T + 3) // 4) * 4
    assert Tcols <= 512, "one PSUM bank holds <= 512 fp32 columns"
    return order, K, B, Tcols


def _groups(Tcols):
    """Group sizes: GCOLS-wide, then a 17-31 column taper of 8s/4s at the
    end.  The wave-A chain is emitted at the first taper group and hides
    under the taper's DMA stream; smaller tail groups also shrink the
    instruction-queue drain after the last transfer.  Groups stay >= 4
    cols so the DMA's contiguous run is >= 512 B (no 2x cost penalty)."""
    sizes = []
    rest = Tcols
    while rest > 2 * GCOLS:
        sizes.append(GCOLS)
        rest -= GCOLS
    n16 = len(sizes)
    while rest > 8:
        sizes.append(8)
        rest -= 8
    if rest:
        sizes.append(rest)   # 4 or 8 (Tcols is a multiple of 4)
    assert sum(sizes) == Tcols and all(s >= 4 for s in sizes), (sizes, Tcols)
    return sizes, n16


def _split(K, B, boundary):
    """Wave split: jA = count of slots whose columns all land before
    `boundary` (the first taper-group column); CA = first wave-B column.
    Guarantees 1 <= jA < SEQS."""
    ends = B + K
    jA = int(np.searchsorted(ends, boundary, side="right"))
    jA = max(1, min(SEQS - 1, jA))
    CA = int(B[jA])
    return jA, CA


def _build_nc(Tcols, K, B, scale=1.0):
    from contextlib import ExitStack

    import concourse.bass as bass
    import concourse.tile as tile
    from concourse import bacc, mybir

    fq = {"f8": mybir.dt.float8e4, "f8e3": mybir.dt.float8e3,
          "f16": mybir.dt.float16}[QDT]
    wcols = 8 if QDT == "f8e3" else 4    # e3m4 sends u as a (hi, lo) pair
    f32 = mybir.dt.float32
    Ttok = Tcols * TILE
    sizes, n16 = _groups(Tcols)
    starts = np.zeros(len(sizes), np.int64)
    starts[1:] = np.cumsum(sizes)[:-1]
    ngrp = len(sizes)
    jA, CA = _split(K, B, int(starts[n16]) if 0 < n16 < ngrp else Tcols)
    nB = SEQS - jA

    # slot j's columns are complete after the group containing B[j]+K[j]-1
    reduces_after = {g: [] for g in range(ngrp)}
    for j in range(SEQS):
        last = int(B[j]) + int(K[j]) - 1
        g = int(np.searchsorted(starts, last, side="right")) - 1
        reduces_after[g].append(j)
    # emit the wave-A chain at the first taper group: its inputs (slots
    # ending before the taper) are ready, and it hides under the taper DMA
    gA_emit = min(n16, ngrp - 1)

    nc = bacc.Bacc("TRN2", target_bir_lowering=False, debug=False,
                   num_devices=NCORES)
    q_h = nc.dram_tensor("qpt", [4, 128, Ttok], fq, kind="ExternalInput")
    w_h = nc.dram_tensor("w", [128, wcols], fq, kind="ExternalInput")
    e_h = nc.dram_tensor("esel", [SEQS, Tcols], f32, kind="ExternalInput")
    eb_h = nc.dram_tensor("eselb", [SEQS, Tcols], f32, kind="ExternalInput")
    out_h = nc.dram_tensor("out", [128, Tcols], f32, kind="ExternalOutput")

    with tile.TileContext(nc) as tc, ExitStack() as ctx:
        singles = ctx.enter_context(tc.tile_pool(name="singles", bufs=1))
        qpool = ctx.enter_context(tc.tile_pool(name="qpool", bufs=QBUFS))
        psum = ctx.enter_context(tc.tile_pool(name="psum", bufs=PSBUFS,
                                              space="PSUM"))
        psum1 = ctx.enter_context(tc.tile_pool(name="psum1", bufs=1,
                                               space="PSUM"))

        # group 0's input DMA goes first so the big stream owns the DMA
        # engines from the start; the small singles hide under it
        qt_first = qpool.tile([128, 4, int(sizes[0]) * TILE], fq,
                              tag=f"qt{int(sizes[0])}")
        nc.sync.dma_start(
            out=qt_first,
            in_=bass.AP(tensor=q_h, offset=0,
                        ap=[[Ttok, 128], [128 * Ttok, 4],
                            [1, int(sizes[0]) * TILE]]))
        w_sb = singles.tile([128, wcols], fq)
        nc.sync.dma_start(out=w_sb, in_=w_h.ap())
        e_sb = singles.tile([SEQS, Tcols], f32)
        nc.sync.dma_start(out=e_sb, in_=e_h.ap())
        eb_sb = singles.tile([SEQS, Tcols], f32)
        nc.sync.dma_start(out=eb_sb, in_=eb_h.ap())
        ones_k = singles.tile([128, 1], f32)
        nc.vector.memset(ones_k, 1.0)
        ones_m = singles.tile([1, 128], f32)
        nc.vector.memset(ones_m, 1.0)

        expm = singles.tile([128, Tcols], f32)
        sums = singles.tile([128, SEQS], f32)
        rec = singles.tile([1, SEQS], f32)
        recbT = singles.tile([SEQS, 128], f32)
        recbTB = singles.tile([SEQS, 128], f32)
        outt = singles.tile([128, Tcols], f32)
        s_ps = psum1.tile([1, SEQS], f32, tag="s_ps")
        r2_ps = psum1.tile([SEQS, 128], f32, tag="r2_ps")
        r2b_ps = psum1.tile([SEQS, 128], f32, tag="r2b_ps")
        sc_ps = psum1.tile([128, Tcols], f32, tag="sc_ps")

        def chain_a():
            nc.tensor.matmul(s_ps[:, :jA], ones_k, sums[:, :jA],
                             start=True, stop=True)
            nc.vector.reciprocal(rec[:, :jA], s_ps[:, :jA])
            nc.tensor.matmul(r2_ps[:jA, :], rec[:, :jA], ones_m,
                             start=True, stop=True)
            nc.vector.tensor_copy(recbT[:jA, :], r2_ps[:jA, :])
            nc.tensor.matmul(sc_ps[:, :CA], recbT[:jA, :], e_sb[:jA, :CA],
                             start=True, stop=True)
            nc.vector.tensor_mul(outt[:, :CA], expm[:, :CA], sc_ps[:, :CA])
            # issue on Activation, not SP: an SP-queued output DMA's sem
            # waits would stall the remaining input DMAs behind it on SP SEQ
            nc.scalar.dma_start(out=out_h.ap()[:, :CA], in_=outt[:, :CA])

        # ---- GEMV + pipelined exp/rowsum, wave-A chain mid-stream
        for g in range(ngrp):
            g0, gn = int(starts[g]), int(sizes[g])
            if g == 0:
                qt = qt_first
            else:
                qt = qpool.tile([128, 4, gn * TILE], fq, tag=f"qt{gn}")
                nc.sync.dma_start(
                    out=qt,
                    in_=bass.AP(tensor=q_h, offset=g0 * TILE,
                                ap=[[Ttok, 128], [128 * Ttok, 4],
                                    [1, gn * TILE]]))
            if g == gA_emit:
                chain_a()
            e_ps = psum.tile([128, 512], f32, tag="eps")
            for tt in range(gn):
                nmm = 0
                for c in range(4):
                    for wc in range(c, wcols, 4):  # hi (and lo) of chunk c
                        nmm += 1
                        nc.tensor.matmul(e_ps[:, tt:tt + 1],
                                         qt[:, c, tt * TILE:(tt + 1) * TILE],
                                         w_sb[:, wc:wc + 1],
                                         start=(nmm == 1), stop=(nmm == wcols))
            nc.scalar.activation(out=expm[:, g0:g0 + gn],
                                 in_=e_ps[:, :gn],
                                 func=mybir.ActivationFunctionType.Exp,
                                 scale=float(scale))
            for j in reduces_after[g]:
                nc.vector.tensor_reduce(out=sums[:, j:j + 1],
                                        in_=expm[:, B[j]:B[j] + K[j]],
                                        axis=mybir.AxisListType.X,
                                        op=mybir.AluOpType.add)

        # ---- wave B: remaining slots, columns [CA, Tcols)
        nc.tensor.matmul(s_ps[:, jA:], ones_k, sums[:, jA:],
                         start=True, stop=True)
        nc.vector.reciprocal(rec[:, jA:], s_ps[:, jA:])
        nc.tensor.matmul(r2b_ps[:nB, :], rec[:, jA:], ones_m,
                         start=True, stop=True)
        nc.vector.tensor_copy(recbTB[:nB, :], r2b_ps[:nB, :])
        nc.tensor.matmul(sc_ps[:, CA:], recbTB[:nB, :], eb_sb[:nB, CA:],
                         start=True, stop=True)
        nc.vector.tensor_mul(outt[:, CA:], expm[:, CA:], sc_ps[:, CA:])
        nc.scalar.dma_start(out=out_h.ap()[:, CA:], in_=outt[:, CA:])

    nc.compile()
    return nc


def _get_nc(Tcols, K, B, scale=1.0):
    key = (Tcols, tuple(int(x) for x in K), float(scale))
    if key not in _nc_cache:
        _nc_cache[key] = _build_nc(Tcols, K, B, scale)
    return _nc_cache[key]


def prepare(questions, questions_lens, lin_w, weight_vec):
    """Host-side sharding: schedule, fold W into u, pack/cast/transpose."""
    import ml_dtypes

    q = np.asarray(questions)
    lens = np.asarray(questions_lens).astype(np.int64).ravel()
    w = np.asarray(lin_w, dtype=np.float64)
    v = np.asarray(weight_vec, dtype=np.float64)
    u = (w.T @ v).astype(np.float32)

    order, K, B, Tcols = _schedule(lens)
    sizes, n16 = _groups(Tcols)
    boundary = (int(np.cumsum(sizes)[n16 - 1])
                if 0 < n16 < len(sizes) else Tcols)
    jA, CA = _split(K, B, boundary)
    Ttok = Tcols * TILE
    unorm = float(u.astype(np.float64) @ u.astype(np.float64))
    if QDT == "f8e3":
        npdt = ml_dtypes.float8_e3m4
        # u scaled by a power of two into e3m4's normal range, sent as a
        # hi+lo pair (residual also e3m4, same scale) -> u error is ~2nd
        # order.  q scaled by QS=2; energies come out scaled by QS*US and
        # the Exp activation un-scales exactly.
        us = 2.0 ** int(np.floor(np.log2(F8MAX / np.abs(u).max())))
        u_s = (u * us).astype(np.float32)
        u_hi = u_s.astype(npdt)
        u_lo = np.clip(u_s - u_hi.astype(np.float32),
                       -F8MAX, F8MAX).astype(npdt)
        scale = 1.0 / (QS * us)
        # pad token: energy ~ -35 (exp ~ 6e-16), capped so QS*pad stays
        # inside e3m4 range
        alpha = min(35.0 / unorm, F8MAX / (QS * float(np.abs(u).max())))
        pad_tok = (-alpha * QS * u).astype(npdt)
        e_pad = float(pad_tok.astype(np.float64)
                      @ (u_hi.astype(np.float64) + u_lo.astype(np.float64))
                      ) * scale
        assert e_pad < -25.0, e_pad
        w_sb = np.ascontiguousarray(np.concatenate(
            [u_hi.reshape(4, 128).T, u_lo.reshape(4, 128).T], axis=1))
        qmax = float(np.abs(q).max())
        qclip = qmax * QS > F8MAX
    else:
        npdt = ml_dtypes.float8_e4m3 if QDT == "f8" else np.float16
        scale = 1.0
        pad_tok = ((-60.0 / unorm) * u).astype(npdt)
        w_sb = np.ascontiguousarray(
            u.reshape(4, 128).T.astype(npdt))      # w_sb[p, c] = u[c*128+p]
    esel = np.zeros((SEQS, Tcols), np.float32)
    for j in range(SEQS):
        esel[j, B[j]:B[j] + K[j]] = 1.0
    eselb = np.zeros((SEQS, Tcols), np.float32)
    eselb[:SEQS - jA] = esel[jA:]                  # wave-B rows, shifted to 0

    in_maps = []
    for c in range(NCORES):
        buf = np.empty((Ttok, EMBED), npdt)
        buf[:] = pad_tok
        for j in range(SEQS):
            r = order[j * NCORES + c]
            n = int(lens[r])
            if QDT == "f8e3":
                row = q[r, :n] * QS
                if qclip:
                    row = np.clip(row, -F8MAX, F8MAX)
                buf[B[j] * TILE:B[j] * TILE + n] = row
            else:
                buf[B[j] * TILE:B[j] * TILE + n] = q[r, :n]
        qpt = np.ascontiguousarray(buf.T).reshape(4, 128, Ttok)
        in_maps.append({"qpt": qpt, "w": w_sb, "esel": esel, "eselb": eselb})
    return in_maps, (order, K, B, Tcols, lens, scale)


def unpack(core_outs, meta):
    order, K, B, Tcols, lens, _scale = meta
    full = np.zeros((B2, LMAX), np.float32)
    for c in range(NCORES):
        o = np.asarray(core_outs[c])                 # [128, Tcols]
        for j in range(SEQS):
            r = order[j * NCORES + c]
            n = int(lens[r])
            blk = o[:, B[j]:B[j] + K[j]]             # [128 tok, K_j tiles]
            full[r, :n] = blk.T.reshape(-1)[:n]
    return full


def run_sharded(questions, questions_lens, lin_w, lin_b, weight_vec,
                trace=False):
    """Shard across the 8 cores, run, gather.  Returns (out, results)."""
    from concourse.bass_utils import run_bass_kernel_spmd

    in_maps, meta = prepare(questions, questions_lens, lin_w, weight_vec)
    nc = _get_nc(meta[3], meta[1], meta[2], meta[5])

    res = None
    last_err = None
    for attempt in range(5):
        try:
            res = run_bass_kernel_spmd(nc, in_maps,
                                       core_ids=list(range(NCORES)),
                                       trace=trace)
            break
        except ModuleNotFoundError:
            trace = False
            continue
        except Exception as e:  # device left unrecoverable by a prior crash
            last_err = e
            if "UNAVAILABLE" in str(e) or "UNRECOVERABLE" in str(e):
                time.sleep(20 * (attempt + 1))
                continue
            raise
    if res is None:
        raise last_err
    out = unpack([r["out"] for r in res.results], meta)
    return out, res


def kernel(questions, questions_lens, lin_w, lin_b, weight_vec):
    out, _ = run_sharded(questions, questions_lens, lin_w, lin_b, weight_vec)
    return out



# revision 25
# speedup vs baseline: 1.9050x; 1.0147x over previous
"""Trainium2 Bass kernel v7: v6 + float8 e3m4 wire format.

Same GEMV/softmax design as v6 (wave-split normalization), but the
questions tensor travels to the device as float8_e3m4 (4 mantissa
bits), halving HBM traffic vs fp16 -- the kernel is DMA-bound, so this
roughly halves exec time.  Numerics: q is scaled by QS=2 and u by a
power of two US so both sit in e3m4's normal range [0.25, 15.5); the
folded weight vector is sent as an e3m4 (hi, lo) pair per embed chunk
(8 accumulating matmuls per 128-token tile instead of 4) so weight
quantization error is second-order; the Exp activation un-scales with
scale=1/(QS*US) (exact, power of two).
"""

import time

import numpy as np

EMBED = 512
LMAX = 2048
NCORES = 8
B2 = 256
SEQS = B2 // NCORES        # 32 sequences per core, one per column-slot
TILE = 128                 # tokens per PSUM column (= stationary cols)
GCOLS = 16                 # columns per DMA group / PSUM bank
QBUFS = 4                  # input tile buffering depth
PSBUFS = 4                 # rotating PSUM banks for the GEMV
QDT = "f8e3"               # wire dtype: "f16", "f8" (e4m3) or "f8e3" (e3m4)
QS = 2.0                   # host-side scale on q before e3m4 cast
F8MAX = 15.49              # e3m4 saturation guard (true max 15.5 -> inf)
WAVE_TAIL = 12             # target wave-B width in columns
EMIT_SLACK = 2             # groups between wave-A data ready and chain emit

_nc_cache = {}


def _schedule(lens):
    """Sort rows by tile count, deal round-robin to cores; every core's
    slot j holds K[j] = max tile count of the 8 rows in that slot."""
    k = (lens + TILE - 1) // TILE            # [256] tiles per row (>=1)
    order = np.argsort(-k, kind="stable")
    K = np.empty(SEQS, np.int64)
    for j in range(SEQS):
        K[j] = k[order[j * NCORES:(j + 1) * NCORES]].max()
    B = np.zeros(SEQS, np.int64)
    B[1:] = np.cumsum(K)[:-1]
    T = int(B[-1] + K[-1])
    Tcols = ((T + 3) // 4) * 4
    assert Tcols <= 512, "one PSUM bank holds <= 512 fp32 columns"
    return order, K, B, Tcols


def _groups(Tcols):
    """Group sizes: GCOLS-wide, then a 17-31 column taper of 8s/4s at the
    end.  The wave-A chain is emitted at the first taper group and hides
    under the taper's DMA stream; smaller tail groups also shrink the
    instruction-queue drain after the last transfer.  Groups stay >= 4
    cols so the DMA's contiguous run is >= 512 B (no 2x cost penalty)."""
    sizes = []
    rest = Tcols
    while rest > 2 * GCOLS:
        sizes.append(GCOLS)
        rest -= GCOLS
    n16 = len(sizes)
    while rest > 8:
        sizes.append(8)
        rest -= 8
    if rest:
        sizes.append(rest)   # 4 or 8 (Tcols is a multiple of 4)
    assert sum(sizes) == Tcols and all(s >= 4 for s in sizes), (sizes, Tcols)
    return sizes, n16


def _split(K, B, boundary):
    """Wave split: jA = count of slots whose columns all land before
    `boundary` (the first taper-group column); CA = first wave-B column.
    Guarantees 1 <= jA < SEQS."""
    ends = B + K
    jA = int(np.searchsorted(ends, boundary, side="right"))
    jA = max(1, min(SEQS - 1, jA))
    CA = int(B[jA])
    return jA, CA


def _build_nc(Tcols, K, B, scale=1.0):
    from contextlib import ExitStack

    import concourse.bass as bass
    import concourse.tile as tile
    from concourse import bacc, mybir

    fq = {"f8": mybir.dt.float8e4, "f8e3": mybir.dt.float8e3,
          "f16": mybir.dt.float16}[QDT]
    wcols = 8 if QDT == "f8e3" else 4    # e3m4 sends u as a (hi, lo) pair
    f32 = mybir.dt.float32
    Ttok = Tcols * TILE
    sizes, n16 = _groups(Tcols)
    starts = np.zeros(len(sizes), np.int64)
    starts[1:] = np.cumsum(sizes)[:-1]
    ngrp = len(sizes)
    jA, CA = _split(K, B, int(starts[n16]) if 0 < n16 < ngrp else Tcols)
    nB = SEQS - jA

    # slot j's columns are complete after the group containing B[j]+K[j]-1
    reduces_after = {g: [] for g in range(ngrp)}
    for j in range(SEQS):
        last = int(B[j]) + int(K[j]) - 1
        g = int(np.searchsorted(starts, last, side="right")) - 1
        reduces_after[g].append(j)
    # emit the wave-A chain at the first taper group: its inputs (slots
    # ending before the taper) are ready, and it hides under the taper DMA
    gA_emit = min(n16, ngrp - 1)

    nc = bacc.Bacc("TRN2", target_bir_lowering=False, debug=False,
                   num_devices=NCORES)
    q_h = nc.dram_tensor("qpt", [4, 128, Ttok], fq, kind="ExternalInput")
    w_h = nc.dram_tensor("w", [128, wcols], fq, kind="ExternalInput")
    e_h = nc.dram_tensor("esel", [SEQS, Tcols], f32, kind="ExternalInput")
    eb_h = nc.dram_tensor("eselb", [SEQS, Tcols], f32, kind="ExternalInput")
    out_h = nc.dram_tensor("out", [128, Tcols], f32, kind="ExternalOutput")

    with tile.TileContext(nc) as tc, ExitStack() as ctx:
        singles = ctx.enter_context(tc.tile_pool(name="singles", bufs=1))
        qpool = ctx.enter_context(tc.tile_pool(name="qpool", bufs=QBUFS))
        psum = ctx.enter_context(tc.tile_pool(name="psum", bufs=PSBUFS,
                                              space="PSUM"))
        psum1 = ctx.enter_context(tc.tile_pool(name="psum1", bufs=1,
                                               space="PSUM"))

        # group 0's input DMA goes first so the big stream owns the DMA
        # engines from the start; the small singles hide under it
        qt_first = qpool.tile([128, 4, int(sizes[0]) * TILE], fq,
                              tag=f"qt{int(sizes[0])}")
        nc.sync.dma_start(
            out=qt_first,
            in_=bass.AP(tensor=q_h, offset=0,
                        ap=[[Ttok, 128], [128 * Ttok, 4],
                            [1, int(sizes[0]) * TILE]]))
        w_sb = singles.tile([128, wcols], fq)
        nc.sync.dma_start(out=w_sb, in_=w_h.ap())
        e_sb = singles.tile([SEQS, Tcols], f32)
        nc.sync.dma_start(out=e_sb, in_=e_h.ap())
        eb_sb = singles.tile([SEQS, Tcols], f32)
        nc.sync.dma_start(out=eb_sb, in_=eb_h.ap())
        ones_k = singles.tile([128, 1], f32)
        nc.vector.memset(ones_k, 1.0)
        ones_m = singles.tile([1, 128], f32)
        nc.vector.memset(ones_m, 1.0)

        expm = singles.tile([128, Tcols], f32)
        sums = singles.tile([128, SEQS], f32)
        ones_sq = singles.tile([SEQS, 128], f32)
        nc.vector.memset(ones_sq, 1.0)
        recTA = singles.tile([SEQS, 1], f32)
        recTB = singles.tile([SEQS, 1], f32)
        rbA = singles.tile([SEQS, 128], f32)
        rbB = singles.tile([SEQS, 128], f32)
        outt = singles.tile([128, Tcols], f32)
        sTA_ps = psum1.tile([SEQS, 1], f32, tag="sTA_ps")
        sTB_ps = psum1.tile([SEQS, 1], f32, tag="sTB_ps")
        sc_ps = psum1.tile([128, Tcols], f32, tag="sc_ps")

        def chain(j0, j1, c0, c1, esel_sb, erow, sT, rT, rbt):
            """Normalize slots [j0, j1) covering columns [c0, c1):
            transposed cross-partition sum -> reciprocal [nj, 1] ->
            per-partition broadcast (tensor_scalar) -> selection matmul ->
            multiply.  The output DMA is issued separately on SP.
            All working tiles are used 0-based (matmul requires base
            partition 0 alignment for small outputs)."""
            nj = j1 - j0
            nc.tensor.matmul(sT[:nj, :], sums[:, j0:j1], ones_k,
                             start=True, stop=True)
            nc.vector.reciprocal(rT[:nj, :], sT[:nj, :])
            nc.vector.tensor_scalar_mul(rbt[:nj, :], ones_sq[:nj, :],
                                        rT[:nj, :])
            nc.tensor.matmul(sc_ps[:, c0:c1], rbt[:nj, :],
                             esel_sb[erow:erow + nj, c0:c1],
                             start=True, stop=True)
            nc.vector.tensor_mul(outt[:, c0:c1], expm[:, c0:c1],
                                 sc_ps[:, c0:c1])

        def chain_a():
            chain(0, jA, 0, CA, e_sb, 0, sTA_ps, recTA, rbA)

        # ---- GEMV + pipelined exp/rowsum, wave-A chain mid-stream
        for g in range(ngrp):
            g0, gn = int(starts[g]), int(sizes[g])
            if g == 0:
                qt = qt_first
            else:
                qt = qpool.tile([128, 4, gn * TILE], fq, tag=f"qt{gn}")
                nc.sync.dma_start(
                    out=qt,
                    in_=bass.AP(tensor=q_h, offset=g0 * TILE,
                                ap=[[Ttok, 128], [128 * Ttok, 4],
                                    [1, gn * TILE]]))
            if g == gA_emit:
                chain_a()
            e_ps = psum.tile([128, 512], f32, tag="eps")
            for tt in range(gn):
                nmm = 0
                for c in range(4):
                    for wc in range(c, wcols, 4):  # hi (and lo) of chunk c
                        nmm += 1
                        nc.tensor.matmul(e_ps[:, tt:tt + 1],
                                         qt[:, c, tt * TILE:(tt + 1) * TILE],
                                         w_sb[:, wc:wc + 1],
                                         start=(nmm == 1), stop=(nmm == wcols))
            nc.scalar.activation(out=expm[:, g0:g0 + gn],
                                 in_=e_ps[:, :gn],
                                 func=mybir.ActivationFunctionType.Exp,
                                 scale=float(scale))
            for j in reduces_after[g]:
                nc.vector.tensor_reduce(out=sums[:, j:j + 1],
                                        in_=expm[:, B[j]:B[j] + K[j]],
                                        axis=mybir.AxisListType.X,
                                        op=mybir.AluOpType.add)

        # ---- output DMAs on SP, emitted after every input DMA so their
        # sem waits can't stall the input stream on SP's SEQ
        nc.sync.dma_start(out=out_h.ap()[:, :CA], in_=outt[:, :CA])

        # ---- wave B: remaining slots, columns [CA, Tcols)
        chain(jA, SEQS, CA, Tcols, eb_sb, 0, sTB_ps, recTB, rbB)
        nc.sync.dma_start(out=out_h.ap()[:, CA:], in_=outt[:, CA:])

    nc.compile()
    return nc


def _get_nc(Tcols, K, B, scale=1.0):
    key = (Tcols, tuple(int(x) for x in K), float(scale))
    if key not in _nc_cache:
        _nc_cache[key] = _build_nc(Tcols, K, B, scale)
    return _nc_cache[key]


def prepare(questions, questions_lens, lin_w, weight_vec):
    """Host-side sharding: schedule, fold W into u, pack/cast/transpose."""
    import ml_dtypes

    q = np.asarray(questions)
    lens = np.asarray(questions_lens).astype(np.int64).ravel()
    w = np.asarray(lin_w, dtype=np.float64)
    v = np.asarray(weight_vec, dtype=np.float64)
    u = (w.T @ v).astype(np.float32)

    order, K, B, Tcols = _schedule(lens)
    sizes, n16 = _groups(Tcols)
    boundary = (int(np.cumsum(sizes)[n16 - 1])
                if 0 < n16 < len(sizes) else Tcols)
    jA, CA = _split(K, B, boundary)
    Ttok = Tcols * TILE
    unorm = float(u.astype(np.float64) @ u.astype(np.float64))
    if QDT == "f8e3":
        npdt = ml_dtypes.float8_e3m4
        # u scaled by a power of two into e3m4's normal range, sent as a
        # hi+lo pair (residual also e3m4, same scale) -> u error is ~2nd
        # order.  q scaled by QS=2; energies come out scaled by QS*US and
        # the Exp activation un-scales exactly.
        us = 2.0 ** int(np.floor(np.log2(F8MAX / np.abs(u).max())))
        u_s = (u * us).astype(np.float32)
        u_hi = u_s.astype(npdt)
        u_lo = np.clip(u_s - u_hi.astype(np.float32),
                       -F8MAX, F8MAX).astype(npdt)
        scale = 1.0 / (QS * us)
        # pad token: energy ~ -35 (exp ~ 6e-16), capped so QS*pad stays
        # inside e3m4 range
        alpha = min(35.0 / unorm, F8MAX / (QS * float(np.abs(u).max())))
        pad_tok = (-alpha * QS * u).astype(npdt)
        e_pad = float(pad_tok.astype(np.float64)
                      @ (u_hi.astype(np.float64) + u_lo.astype(np.float64))
                      ) * scale
        assert e_pad < -25.0, e_pad
        w_sb = np.ascontiguousarray(np.concatenate(
            [u_hi.reshape(4, 128).T, u_lo.reshape(4, 128).T], axis=1))
        qmax = float(np.abs(q).max())
        qclip = qmax * QS > F8MAX
    else:
        npdt = ml_dtypes.float8_e4m3 if QDT == "f8" else np.float16
        scale = 1.0
        pad_tok = ((-60.0 / unorm) * u).astype(npdt)
        w_sb = np.ascontiguousarray(
            u.reshape(4, 128).T.astype(npdt))      # w_sb[p, c] = u[c*128+p]
    esel = np.zeros((SEQS, Tcols), np.float32)
    for j in range(SEQS):
        esel[j, B[j]:B[j] + K[j]] = 1.0
    eselb = np.zeros((SEQS, Tcols), np.float32)
    eselb[:SEQS - jA] = esel[jA:]                  # wave-B rows, shifted to 0

    in_maps = []
    for c in range(NCORES):
        buf = np.empty((Ttok, EMBED), npdt)
        buf[:] = pad_tok
        for j in range(SEQS):
            r = order[j * NCORES + c]
            n = int(lens[r])
            if QDT == "f8e3":
                row = q[r, :n] * QS
                if qclip:
                    row = np.clip(row, -F8MAX, F8MAX)
                buf[B[j] * TILE:B[j] * TILE + n] = row
            else:
                buf[B[j] * TILE:B[j] * TILE + n] = q[r, :n]
        qpt = np.ascontiguousarray(buf.T).reshape(4, 128, Ttok)
        in_maps.append({"qpt": qpt, "w": w_sb, "esel": esel, "eselb": eselb})
    return in_maps, (order, K, B, Tcols, lens, scale)


def unpack(core_outs, meta):
    order, K, B, Tcols, lens, _scale = meta
    full = np.zeros((B2, LMAX), np.float32)
    for c in range(NCORES):
        o = np.asarray(core_outs[c])                 # [128, Tcols]
        for j in range(SEQS):
            r = order[j * NCORES + c]
            n = int(lens[r])
            blk = o[:, B[j]:B[j] + K[j]]             # [128 tok, K_j tiles]
            full[r, :n] = blk.T.reshape(-1)[:n]
    return full


def run_sharded(questions, questions_lens, lin_w, lin_b, weight_vec,
                trace=False):
    """Shard across the 8 cores, run, gather.  Returns (out, results)."""
    from concourse.bass_utils import run_bass_kernel_spmd

    in_maps, meta = prepare(questions, questions_lens, lin_w, weight_vec)
    nc = _get_nc(meta[3], meta[1], meta[2], meta[5])

    res = None
    last_err = None
    for attempt in range(5):
        try:
            res = run_bass_kernel_spmd(nc, in_maps,
                                       core_ids=list(range(NCORES)),
                                       trace=trace)
            break
        except ModuleNotFoundError:
            trace = False
            continue
        except Exception as e:  # device left unrecoverable by a prior crash
            last_err = e
            if "UNAVAILABLE" in str(e) or "UNRECOVERABLE" in str(e):
                time.sleep(20 * (attempt + 1))
                continue
            raise
    if res is None:
        raise last_err
    out = unpack([r["out"] for r in res.results], meta)
    return out, res


def kernel(questions, questions_lens, lin_w, lin_b, weight_vec):
    out, _ = run_sharded(questions, questions_lens, lin_w, lin_b, weight_vec)
    return out



# revision 27
# speedup vs baseline: 1.9692x; 1.0337x over previous
"""Trainium2 Bass kernel v9: v8 + exact token packing.

GEMV/softmax design as before (f8e3 wire, wave-split normalization),
but sequences now pack back-to-back at token granularity instead of
128-token tiles: slot j starts at shared token offset s_j and has the
shared budget L_j = max length of its 8 rows (one per core).  A PSUM
column (128 tokens) can then straddle two slots; such shared boundary
columns are handled with per-partition masks:

- row sums: sum_j = interior-column reduce + mE.CB (own end partial)
  + (CB - mE.CB) shifted (start partial), folded into the existing
  cross-partition sum as three accumulating matmuls;
- outputs: the rec-selection matmul gains two mask-weighted terms
  (rb*mET @ eselE and rb*mST @ eselS) in the same PSUM accumulation,
  so boundary columns get the right reciprocal per partition with no
  per-column scatter.

This shrinks the DMA stream from 284 to ~268 columns (~3 us).
"""

import time

import numpy as np

EMBED = 512
LMAX = 2048
NCORES = 8
B2 = 256
SEQS = B2 // NCORES        # 32 sequences per core, one per column-slot
TILE = 128                 # tokens per PSUM column
GCOLS = 16                 # columns per DMA group / PSUM bank
QBUFS = 4                  # input tile buffering depth
PSBUFS = 4                 # rotating PSUM banks for the GEMV
QDT = "f8e3"               # wire dtype: "f16", "f8" (e4m3) or "f8e3" (e3m4)
QS = 2.0                   # host-side scale on q before e3m4 cast
F8MAX = 15.49              # e3m4 saturation guard (true max 15.5 -> inf)

_nc_cache = {}


def _schedule(lens):
    """Sort rows by length, deal round-robin to cores; slot j's shared
    budget L[j] = max length of the 8 rows in that slot (>= TILE so a
    column never straddles more than two slots)."""
    order = np.argsort(-lens, kind="stable")
    L = np.maximum(lens[order[np.arange(SEQS) * NCORES]], TILE)
    return order, L.astype(np.int64)


def _derive(L):
    """Shared packing geometry from the slot budgets."""
    L = np.asarray(L, np.int64)
    s = np.zeros(SEQS, np.int64)
    s[1:] = np.cumsum(L)[:-1]
    N = int(s[-1] + L[-1])
    Tcols = ((-(-N // TILE)) + 3) // 4 * 4
    assert Tcols <= 512, "one PSUM bank holds <= 512 fp32 columns"
    e = s + L
    c0 = s // TILE
    c1 = (e - 1) // TILE
    ef = e % TILE
    shared = (ef != 0) & (np.arange(SEQS) < SEQS - 1)  # end col shared w/ next
    i0 = c0 + (s % TILE != 0)
    i1 = np.where(shared, e // TILE - 1, c1)  # unshared end col is interior
    return s, Tcols, c0, c1, ef, shared, i0, i1


def _groups(Tcols):
    """16-wide DMA groups then an 8s/4 taper (17-32 cols) at the end; the
    wave-A chain hides under the taper stream and small tail groups keep
    the post-stream instruction drain short.  Groups >= 4 cols keep the
    DMA's contiguous run >= 512 B (no 2x cost penalty)."""
    sizes = []
    rest = Tcols
    while rest > 2 * GCOLS:
        sizes.append(GCOLS)
        rest -= GCOLS
    n16 = len(sizes)
    while rest > 8:
        sizes.append(8)
        rest -= 8
    if rest:
        sizes.append(rest)
    assert sum(sizes) == Tcols and all(s >= 4 for s in sizes), (sizes, Tcols)
    return sizes, n16


def _split(c1, boundary):
    """jA = count of slots whose last column lands before the taper."""
    jA = int((np.asarray(c1) < boundary).sum())
    return max(1, min(SEQS - 1, jA))


def _build_nc(L, scale):
    from contextlib import ExitStack

    import concourse.bass as bass
    import concourse.tile as tile
    from concourse import bacc, mybir

    fq = {"f8": mybir.dt.float8e4, "f8e3": mybir.dt.float8e3,
          "f16": mybir.dt.float16}[QDT]
    wcols = 8 if QDT == "f8e3" else 4    # e3m4 sends u as a (hi, lo) pair
    f32 = mybir.dt.float32
    f16 = mybir.dt.float16
    sub = mybir.AluOpType.subtract

    s, Tcols, c0, c1, ef, shared, i0, i1 = _derive(L)
    Ttok = Tcols * TILE
    sizes, n16 = _groups(Tcols)
    starts = np.zeros(len(sizes), np.int64)
    starts[1:] = np.cumsum(sizes)[:-1]
    ngrp = len(sizes)
    jA = _split(c1, int(starts[n16]) if 0 < n16 < ngrp else Tcols)
    jB0 = jA - 1          # wave B re-covers slot jA-1 (shared col w/ jA)
    CA = int(c0[jA])
    nB = SEQS - jB0

    grp_of = lambda col: int(np.searchsorted(starts, col, side="right")) - 1
    reduces_after = {g: [] for g in range(ngrp)}
    cb_after = {g: [] for g in range(ngrp)}
    for j in range(SEQS):
        if i1[j] >= i0[j]:
            reduces_after[grp_of(int(i1[j]))].append(j)
        if shared[j]:
            cb_after[grp_of(int(c1[j]))].append(j)
    gA_emit = min(n16, ngrp - 1)

    nc = bacc.Bacc("TRN2", target_bir_lowering=False, debug=False,
                   num_devices=NCORES)
    q_h = nc.dram_tensor("qpt", [4, 128, Ttok], fq, kind="ExternalInput")
    w_h = nc.dram_tensor("w", [128, wcols], fq, kind="ExternalInput")
    # eselpk rows 0-2: wave-A interior/end/start selectors; rows 3-5: the
    # same shifted down by jB0 for wave B
    e_h = nc.dram_tensor("eselpk", [SEQS, 6, Tcols], f16,
                         kind="ExternalInput")
    # mskpk: mET, mST (wave A) and their jB0-shifted variants (wave B)
    m_h = nc.dram_tensor("mskpk", [SEQS, 4, 128], f16, kind="ExternalInput")
    me_h = nc.dram_tensor("me", [128, SEQS], f16, kind="ExternalInput")
    out_h = nc.dram_tensor("out", [128, Tcols], f32, kind="ExternalOutput")

    with tile.TileContext(nc) as tc, ExitStack() as ctx:
        singles = ctx.enter_context(tc.tile_pool(name="singles", bufs=1))
        qpool = ctx.enter_context(tc.tile_pool(name="qpool", bufs=QBUFS))
        psum = ctx.enter_context(tc.tile_pool(name="psum", bufs=PSBUFS,
                                              space="PSUM"))
        psum1 = ctx.enter_context(tc.tile_pool(name="psum1", bufs=1,
                                               space="PSUM"))

        # group 0's input DMA goes first so the big stream owns the DMA
        # engines from the start; the small singles hide under it
        qt_first = qpool.tile([128, 4, int(sizes[0]) * TILE], fq,
                              tag=f"qt{int(sizes[0])}")
        nc.sync.dma_start(
            out=qt_first,
            in_=bass.AP(tensor=q_h, offset=0,
                        ap=[[Ttok, 128], [128 * Ttok, 4],
                            [1, int(sizes[0]) * TILE]]))
        w_sb = singles.tile([128, wcols], fq)
        nc.sync.dma_start(out=w_sb, in_=w_h.ap())
        e_sb = singles.tile([SEQS, 6, Tcols], f16)
        nc.sync.dma_start(out=e_sb, in_=e_h.ap())
        m_sb = singles.tile([SEQS, 4, 128], f16)
        nc.sync.dma_start(out=m_sb, in_=m_h.ap())
        me_sb = singles.tile([128, SEQS], f16)
        nc.sync.dma_start(out=me_sb, in_=me_h.ap())
        ones_k = singles.tile([128, 1], f32)
        nc.vector.memset(ones_k, 1.0)
        ones_sq = singles.tile([SEQS, 128], f32)
        nc.vector.memset(ones_sq, 1.0)

        expm = singles.tile([128, Tcols], f32)
        sums = singles.tile([128, SEQS], f32)
        nc.gpsimd.memset(sums, 0.0)
        CB = singles.tile([128, SEQS], f16)       # gathered boundary columns
        nc.gpsimd.memset(CB, 0.0)
        Q1 = singles.tile([128, SEQS], f32)       # mE * CB (own end partial)
        Q2z = singles.tile([128, SEQS], f32)      # (CB - Q1) shifted right
        nc.gpsimd.memset(Q2z, 0.0)
        recTA = singles.tile([SEQS, 1], f32)
        recTB = singles.tile([SEQS, 1], f32)
        rbA = singles.tile([SEQS, 128], f16)
        rbB = singles.tile([SEQS, 128], f16)
        rbWA = singles.tile([SEQS, 128], f16)
        rbWB = singles.tile([SEQS, 128], f16)
        rbW2A = singles.tile([SEQS, 128], f16)
        rbW2B = singles.tile([SEQS, 128], f16)
        outt = singles.tile([128, Tcols], f32)
        sTA_ps = psum1.tile([SEQS, 1], f32, tag="sTA_ps")
        sTB_ps = psum1.tile([SEQS, 1], f32, tag="sTB_ps")
        sc_ps = psum1.tile([128, Tcols], f32, tag="sc_ps")

        def chain(j0, ca, cb, erow, sT, rT, rbt, rbWt, rbW2t, mrow):
            """Normalize slots [j0, SEQS-ish) covering columns [ca, cb):
            boundary partials -> 3-way accumulated transposed sum ->
            reciprocal -> per-partition broadcast -> mask-weighted 3-way
            selection matmul -> multiply.  Output DMA issued separately."""
            j1 = jA if j0 == 0 else SEQS
            nj = j1 - j0
            nc.vector.tensor_mul(Q1[:, j0:j1], me_sb[:, j0:j1],
                                 CB[:, j0:j1])
            lo = max(j0, 1)
            nc.vector.tensor_sub(Q2z[:, lo:j1], CB[:, lo - 1:j1 - 1],
                                 Q1[:, lo - 1:j1 - 1])
            nc.tensor.matmul(sT[:nj, :], sums[:, j0:j1], ones_k,
                             start=True, stop=False)
            nc.tensor.matmul(sT[:nj, :], Q1[:, j0:j1], ones_k,
                             start=False, stop=False)
            nc.tensor.matmul(sT[:nj, :], Q2z[:, j0:j1], ones_k,
                             start=False, stop=True)
            nc.vector.reciprocal(rT[:nj, :], sT[:nj, :])
            nc.vector.tensor_scalar_mul(rbt[:nj, :], ones_sq[:nj, :],
                                        rT[:nj, :])
            nc.vector.tensor_mul(rbWt[:nj, :], rbt[:nj, :],
                                 m_sb[:nj, mrow, :])
            nc.vector.tensor_mul(rbW2t[:nj, :], rbt[:nj, :],
                                 m_sb[:nj, mrow + 1, :])
            nc.tensor.matmul(sc_ps[:, ca:cb], rbt[:nj, :],
                             e_sb[erow:erow + nj, 0 if j0 == 0 else 3,
                                  ca:cb],
                             start=True, stop=False)
            nc.tensor.matmul(sc_ps[:, ca:cb], rbWt[:nj, :],
                             e_sb[erow:erow + nj, 1 if j0 == 0 else 4,
                                  ca:cb],
                             start=False, stop=False)
            nc.tensor.matmul(sc_ps[:, ca:cb], rbW2t[:nj, :],
                             e_sb[erow:erow + nj, 2 if j0 == 0 else 5,
                                  ca:cb],
                             start=False, stop=True)
            nc.vector.tensor_mul(outt[:, ca:cb], expm[:, ca:cb],
                                 sc_ps[:, ca:cb])

        def chain_a():
            chain(0, 0, CA, 0, sTA_ps, recTA, rbA, rbWA, rbW2A, 0)

        # ---- GEMV + pipelined exp/rowsum, wave-A chain mid-stream
        for g in range(ngrp):
            g0, gn = int(starts[g]), int(sizes[g])
            if g == 0:
                qt = qt_first
            else:
                qt = qpool.tile([128, 4, gn * TILE], fq, tag=f"qt{gn}")
                nc.sync.dma_start(
                    out=qt,
                    in_=bass.AP(tensor=q_h, offset=g0 * TILE,
                                ap=[[Ttok, 128], [128 * Ttok, 4],
                                    [1, gn * TILE]]))
            if g == gA_emit:
                chain_a()
            e_ps = psum.tile([128, 512], f32, tag="eps")
            for tt in range(gn):
                nmm = 0
                for c in range(4):
                    for wc in range(c, wcols, 4):  # hi (and lo) of chunk c
                        nmm += 1
                        nc.tensor.matmul(e_ps[:, tt:tt + 1],
                                         qt[:, c, tt * TILE:(tt + 1) * TILE],
                                         w_sb[:, wc:wc + 1],
                                         start=(nmm == 1), stop=(nmm == wcols))
            nc.scalar.activation(out=expm[:, g0:g0 + gn],
                                 in_=e_ps[:, :gn],
                                 func=mybir.ActivationFunctionType.Exp,
                                 scale=float(scale))
            for j in cb_after[g]:
                nc.vector.tensor_copy(CB[:, j:j + 1],
                                      expm[:, int(c1[j]):int(c1[j]) + 1])
            for j in reduces_after[g]:
                nc.vector.tensor_reduce(out=sums[:, j:j + 1],
                                        in_=expm[:, int(i0[j]):int(i1[j]) + 1],
                                        axis=mybir.AxisListType.X,
                                        op=mybir.AluOpType.add)

        # ---- output DMAs on SP, emitted after every input DMA so their
        # sem waits can't stall the input stream on SP's SEQ
        nc.sync.dma_start(out=out_h.ap()[:, :CA], in_=outt[:, :CA])

        # ---- wave B: slots [jB0, SEQS), columns [CA, Tcols)
        chain(jB0, CA, Tcols, 0, sTB_ps, recTB, rbB, rbWB, rbW2B, 2)
        nc.sync.dma_start(out=out_h.ap()[:, CA:], in_=outt[:, CA:])

    nc.compile()
    return nc


def _get_nc(L, scale):
    key = (tuple(int(x) for x in L), float(scale))
    if key not in _nc_cache:
        _nc_cache[key] = _build_nc(np.asarray(L, np.int64), scale)
    return _nc_cache[key]


def prepare(questions, questions_lens, lin_w, weight_vec):
    """Host-side sharding: schedule, fold W into u, pack/cast/scale."""
    import ml_dtypes

    q = np.asarray(questions)
    lens = np.asarray(questions_lens).astype(np.int64).ravel()
    w = np.asarray(lin_w, dtype=np.float64)
    v = np.asarray(weight_vec, dtype=np.float64)
    u = (w.T @ v).astype(np.float32)

    order, L = _schedule(lens)
    s, Tcols, c0, c1, ef, shared, i0, i1 = _derive(L)
    sizes, n16 = _groups(Tcols)
    jA = _split(c1, int(np.cumsum(sizes)[n16 - 1])
                if 0 < n16 < len(sizes) else Tcols)
    jB0 = jA - 1
    Ttok = Tcols * TILE
    unorm = float(u.astype(np.float64) @ u.astype(np.float64))

    assert QDT == "f8e3"
    npdt = ml_dtypes.float8_e3m4
    us = 2.0 ** int(np.floor(np.log2(F8MAX / np.abs(u).max())))
    u_s = (u * us).astype(np.float32)
    u_hi = u_s.astype(npdt)
    u_lo = np.clip(u_s - u_hi.astype(np.float32), -F8MAX, F8MAX).astype(npdt)
    scale = 1.0 / (QS * us)
    alpha = min(35.0 / unorm, F8MAX / (QS * float(np.abs(u).max())))
    pad_tok = (-alpha * QS * u).astype(npdt)
    e_pad = float(pad_tok.astype(np.float64)
                  @ (u_hi.astype(np.float64) + u_lo.astype(np.float64))
                  ) * scale
    assert e_pad < -25.0, e_pad
    w_sb = np.ascontiguousarray(np.concatenate(
        [u_hi.reshape(4, 128).T, u_lo.reshape(4, 128).T], axis=1))
    qmax = float(np.abs(q).max())
    qclip = qmax * QS > F8MAX

    # selection matrices: interior cols, shared-end col, shared-start col
    esel = np.zeros((SEQS, Tcols), np.float16)
    eselE = np.zeros((SEQS, Tcols), np.float16)
    eselS = np.zeros((SEQS, Tcols), np.float16)
    for j in range(SEQS):
        if i1[j] >= i0[j]:
            esel[j, i0[j]:i1[j] + 1] = 1.0
        if shared[j]:
            eselE[j, c1[j]] = 1.0
        if j > 0 and shared[j - 1]:
            eselS[j, c0[j]] = 1.0
    eselpk = np.zeros((SEQS, 6, Tcols), np.float16)
    eselpk[:, 0] = esel
    eselpk[:, 1] = eselE
    eselpk[:, 2] = eselS
    eselpk[:SEQS - jB0, 3] = esel[jB0:]
    eselpk[:SEQS - jB0, 4] = eselE[jB0:]
    eselpk[:SEQS - jB0, 5] = eselS[jB0:]

    # partition masks: mE[p, j] = 1 iff shared[j] and p < ef[j]
    mE = np.zeros((128, SEQS), np.float16)
    for j in range(SEQS):
        if shared[j]:
            mE[:int(ef[j]), j] = 1.0
    mET = np.ascontiguousarray(mE.T)
    mST = np.zeros((SEQS, 128), np.float16)
    mST[1:] = 1.0 - mET[:-1]
    for j in range(1, SEQS):
        if not shared[j - 1]:
            mST[j] = 0.0
    mskpk = np.zeros((SEQS, 4, 128), np.float16)
    mskpk[:, 0] = mET
    mskpk[:, 1] = mST
    mskpk[:SEQS - jB0, 2] = mET[jB0:]
    mskpk[:SEQS - jB0, 3] = mST[jB0:]

    in_maps = []
    for c in range(NCORES):
        buf = np.empty((Ttok, EMBED), npdt)
        buf[:] = pad_tok
        for j in range(SEQS):
            r = order[j * NCORES + c]
            n = int(lens[r])
            row = q[r, :n] * QS
            if qclip:
                row = np.clip(row, -F8MAX, F8MAX)
            buf[s[j]:s[j] + n] = row
        qpt = np.ascontiguousarray(buf.T).reshape(4, 128, Ttok)
        in_maps.append({"qpt": qpt, "w": w_sb, "eselpk": eselpk,
                        "mskpk": mskpk, "me": mE})
    return in_maps, (order, L, s, Tcols, lens, scale)


def unpack(core_outs, meta):
    order, L, s, Tcols, lens, _scale = meta
    full = np.zeros((B2, LMAX), np.float32)
    for c in range(NCORES):
        flat = np.asarray(core_outs[c]).T.reshape(-1)   # token-major
        for j in range(SEQS):
            r = order[j * NCORES + c]
            n = int(lens[r])
            full[r, :n] = flat[s[j]:s[j] + n]
    return full


def run_sharded(questions, questions_lens, lin_w, lin_b, weight_vec,
                trace=False):
    """Shard across the 8 cores, run, gather.  Returns (out, results)."""
    from concourse.bass_utils import run_bass_kernel_spmd

    in_maps, meta = prepare(questions, questions_lens, lin_w, weight_vec)
    nc = _get_nc(meta[1], meta[5])

    res = None
    last_err = None
    for attempt in range(5):
        try:
            res = run_bass_kernel_spmd(nc, in_maps,
                                       core_ids=list(range(NCORES)),
                                       trace=trace)
            break
        except ModuleNotFoundError:
            trace = False
            continue
        except Exception as e:  # device left unrecoverable by a prior crash
            last_err = e
            if "UNAVAILABLE" in str(e) or "UNRECOVERABLE" in str(e):
                time.sleep(20 * (attempt + 1))
                continue
            raise
    if res is None:
        raise last_err
    out = unpack([r["out"] for r in res.results], meta)
    return out, res


def kernel(questions, questions_lens, lin_w, lin_b, weight_vec):
    out, _ = run_sharded(questions, questions_lens, lin_w, lin_b, weight_vec)
    return out
